# revision 13
# baseline (speedup 1.0000x reference)
"""Trainium2 Bass kernel: 3x3 VALID conv2d, stride 1.

Full input [32, 64, 112, 112] f32 + weights [128, 64, 3, 3] f32
-> output [32, 128, 110, 110] f32.

Data-parallel across 8 NeuronCores: 4 images per core.

Per-core formulation: conv as PE matmuls, out = lhsT.T @ rhs with
K (contraction, partitions) = (row-shift s in {0,1}) x (64 channels) = 128,
M (out partitions) = 128 output channels,
N (moving free dim) = 4 input-width rows = 448 (<= 512, one PSUM bank).
The 2 rightmost columns of each 112-wide row are conv garbage; the
PSUM->SBUF copy compacts to the valid 110 columns. A fully contiguous
rhs stream keeps the PE moving-operand path at full rate.

Each image lives in SBUF twice: copy A = rows 0..111 in partitions 0..63,
copy B = rows 1..111 (shifted up one row) in partitions 64..127, built by
an SBUF->SBUF DMA from copy A. A single K=128 matmul against weight plane
kx then applies taps (ky=0, kx) and (ky=1, kx) at once; ky=2 taps reuse
the same tile at +1 row with weights zero-padded on the A half. 6 matmuls
per chunk instead of 9.

Inputs are cast to fp16 on the host: fp16 operands stream the PE at full
rate (fp32 is 4x slower, fp32r 2x), with fp32 PSUM accumulation the rel
err is ~3e-4. Casting host-side halves HBM input traffic and avoids the
slow SWDGE cast-DMA path (~147 GB/s measured).

Schedule: chunks are processed in groups of 8 across the 8 PSUM banks,
weight-plane-major (m outer), so consecutive matmuls hit different banks
(drain overlaps fill) and reuse the same stationary weights.
"""

import numpy as np

B_FULL = 32
N_CORES = 8
B_CORE = B_FULL // N_CORES  # 4 images per core
C_IN = 64
C_OUT = 128
H = W = 112
OH = OW = 110

_NC = None


def _chunks():
    # per image: 27 chunks of 4 output rows + 1 of 2 rows = 110
    rows_list = [4] * 27 + [2]
    out = []
    for b in range(B_CORE):
        y0 = 0
        for r in rows_list:
            out.append((b, y0, r))
            y0 += r
        assert y0 == OH
    return out


def _build():
    from contextlib import ExitStack

    import concourse.tile as tile
    from concourse import bacc, mybir

    nc = bacc.Bacc("TRN2", target_bir_lowering=False, debug=False)
    # host-duplicated layout: [b, s*64+ci, h*112+w] with s=0 -> row h,
    # s=1 -> row h+1 (see kernel()); full 128-partition DMAs use all 16
    # SBUF ports and need no on-device shift copies
    x = nc.dram_tensor(
        "x", [B_CORE, 128, H * W], mybir.dt.float16, kind="ExternalInput"
    )
    w = nc.dram_tensor("w", [128, 6, 128], mybir.dt.float16, kind="ExternalInput")
    y = nc.dram_tensor(
        "y", [B_CORE, C_OUT, OH, OW], mybir.dt.float32, kind="ExternalOutput"
    )

    chunks = _chunks()
    assert len(chunks) % 8 == 0
    n_groups = len(chunks) // 8

    with tile.TileContext(nc) as tc, ExitStack() as ctx:
        xpool = ctx.enter_context(tc.tile_pool(name="xp", bufs=4))
        wpool = ctx.enter_context(tc.tile_pool(name="wp", bufs=1))
        opool = ctx.enter_context(tc.tile_pool(name="op", bufs=8))
        ppool = ctx.enter_context(tc.tile_pool(name="pp", bufs=8, space="PSUM"))

        wt = wpool.tile([128, 6, 128], mybir.dt.float16)
        nc.sync.dma_start(wt[:], w.ap())

        xa = x.ap()
        ya = y.ap()

        # 3 row bands per image so the first chunks start early
        BANDS = [0, 8, 34, 61, 87, H]
        xtiles = []
        for b in range(B_CORE):
            xt = xpool.tile([128, H * W], mybir.dt.float16, tag="xt")
            for lo, hi in zip(BANDS, BANDS[1:]):
                nc.gpsimd.dma_start(
                    xt[:, lo * W : hi * W], xa[b][:, lo * W : hi * W]
                )
            xtiles.append(xt)

        for g in range(n_groups):
            gchunks = chunks[g * 8 : (g + 1) * 8]
            pts = [
                ppool.tile([128, 448], mybir.dt.float32, name="pt", tag="pt")
                for _ in range(8)
            ]
            for m in range(6):
                kx = m % 3
                dy = 0 if m < 3 else 1
                for j, (b, y0, rows) in enumerate(gchunks):
                    n = rows * W
                    rhs = xtiles[b][:, (y0 + dy) * W + kx : (y0 + dy) * W + kx + n]
                    nc.tensor.matmul(
                        pts[j][:, 0:n],
                        wt[:, m, :],
                        rhs,
                        start=(m == 0),
                        stop=(m == 5),
                        skip_group_check=True,
                    )
            # batch outputs per 4-chunk half: one contiguous ~0.9MB DMA each
            # (small per-chunk DMAs run at ~150 GB/s, descriptor-dominated)
            for h in range(2):
                hchunks = gchunks[4 * h : 4 * h + 4]
                total_rows = sum(r for _, _, r in hchunks)
                ot = opool.tile([128, 16 * OW], mybir.dt.float32, tag="ot")
                off = 0
                for jj, (b, y0, rows) in enumerate(hchunks):
                    j = 4 * h + jj
                    # compact 112-wide psum rows to the 110 valid columns
                    psrc = pts[j][:].rearrange("p (r c) -> p r c", c=W)[
                        :, 0:rows, 0:OW
                    ]
                    odst = ot[:, off : off + rows * OW].rearrange(
                        "p (r c) -> p r c", c=OW
                    )
                    if j % 2 == 0:
                        nc.vector.tensor_copy(odst, psrc)
                    else:
                        nc.scalar.copy(odst, psrc)
                    off += rows * OW
                b0, y00, _ = hchunks[0]
                assert all(b == b0 for b, _, _ in hchunks)
                assert hchunks[-1][1] + hchunks[-1][2] - y00 == total_rows
                nc.sync.dma_start(
                    ya[b0].rearrange("c h w -> c (h w)")[
                        :, y00 * OW : y00 * OW + total_rows * OW
                    ],
                    ot[:, 0 : total_rows * OW],
                )

    nc.compile()
    return nc


def _get_nc():
    global _NC
    if _NC is None:
        _NC = _build()
    return _NC


def _prep_weights(weights: np.ndarray) -> np.ndarray:
    # w6[s*64+ci, kx, co]   = w[co, ci, ky=s, kx]   (fused ky=0/1 planes)
    # w6[64+ci,  3+kx, co]  = w[co, ci, ky=2, kx]   (ky=2 planes, A half zero)
    w = np.asarray(weights, dtype=np.float32)
    wt = w.transpose(1, 2, 3, 0)  # [ci, ky, kx, co]
    w6 = np.zeros((128, 6, 128), np.float32)
    w6[0:64, 0:3, :] = wt[:, 0, :, :]
    w6[64:128, 0:3, :] = wt[:, 1, :, :]
    w6[64:128, 3:6, :] = wt[:, 2, :, :]
    return w6.astype(np.float16)


def kernel(input_image: np.ndarray, weights: np.ndarray, _trace: bool = False):
    from concourse.bass_utils import run_bass_kernel_spmd

    nc = _get_nc()
    x16 = np.asarray(input_image).astype(np.float16)  # [32, 64, 112, 112]
    xd = np.zeros((B_FULL, 128, H * W), np.float16)
    xd[:, :C_IN] = x16.reshape(B_FULL, C_IN, H * W)
    xd[:, C_IN:, : (H - 1) * W] = x16[:, :, 1:, :].reshape(B_FULL, C_IN, -1)
    w6 = _prep_weights(weights)
    in_maps = [
        {"x": xd[B_CORE * i : B_CORE * (i + 1)], "w": w6} for i in range(N_CORES)
    ]
    res = run_bass_kernel_spmd(
        nc, in_maps, core_ids=list(range(N_CORES)), trace=_trace
    )
    out = np.concatenate([res.results[i]["y"] for i in range(N_CORES)], axis=0)
    if _trace:
        return out, res
    return out
